# revision 3
# baseline (speedup 1.0000x reference)
"""Causal self-attention (T=4096, C=2048, 16 heads) on 8 TRN2 NeuronCores.

Sharding: tensor-parallel over heads (2 heads/core) for QKV + attention,
then per-head AllToAlls redistribute the attention output to
token-parallel (512 tokens/core) for the output projection. No reduction
collective is needed: each core computes full output rows for its token
slice and the host concatenates.

Precision split:
  - QKV projection and QK^T scores run as float32r (rounded fp32,
    ~1e-4 relative) - these feed exp(), which amplifies absolute score
    error, so they stay high precision.
  - exp(P), P@V, softmax denominator, and the output projection run in
    bf16 (the softmax normalization and the 2e-2 gate tolerate ~1e-3).
Scores are computed transposed (keys on partitions, queries free) so
softmax denominators come from a ones-vector matmul, P@V needs no
transposes, and causal masking touches only diagonal tiles via
affine_select; upper-triangle blocks are skipped entirely.
"""
import sys
import types

sys.path.insert(0, "/opt/trn_rl_repo")

import numpy as np

from concourse import bacc, tile
import concourse.mybir as mybir
from concourse.bass_utils import run_bass_kernel_spmd

F32 = mybir.dt.float32
F32R = mybir.dt.float32r
BF16 = mybir.dt.bfloat16

T, C = 4096, 2048
H, D = 16, 128
W = 8                  # cores
HL = H // W            # heads per core (2)
CL = HL * D            # local attention-output columns (256)
KT = C // 128          # contraction tiles (16)
TC1 = 512              # phase-1 token chunk
NC1 = T // TC1         # 8
TC2 = 512              # phase-2/3 token chunk
NC2 = T // TC2         # 8
TL = T // W            # tokens per core for the projection (512)
SCALE = float(1.0 / np.sqrt(D))

TRACE = False          # test harness sets kernel.TRACE = True for profiling
LAST_RESULT = {}       # test harness reads exec_time_ns from here

_cache = {}


def _build():
    nc = bacc.Bacc("TRN2", target_bir_lowering=False, debug=False, num_devices=W)
    xT_d = nc.dram_tensor("xT", [C, T], F32, kind="ExternalInput")
    wqkT_d = nc.dram_tensor("wqkT", [C, 2 * CL], F32, kind="ExternalInput")
    wvT_d = nc.dram_tensor("wvT", [C, CL], F32, kind="ExternalInput")
    wpT_d = nc.dram_tensor("wpT", [C, C], F32, kind="ExternalInput")
    out_d = nc.dram_tensor("out", [TL, C], F32, kind="ExternalOutput")

    with tile.TileContext(nc) as tc:
        with tc.tile_pool(name="res", bufs=1) as res, \
             tc.tile_pool(name="dram", bufs=1, space="DRAM") as dram:
            # per-head A2A buffers (bf16): shard j = my token chunk j
            a2a_in = [dram.tile([W, 128, TC2], BF16, tag=f"a2a_in{h}",
                                name=f"a2a_in{h}") for h in range(HL)]
            a2a_out = [dram.tile([W, 128, TC2], BF16, tag=f"a2a_out{h}",
                                 name=f"a2a_out{h}") for h in range(HL)]

            # resident q/k (transposed, [d, t], f32r) and V ([s, d], bf16)
            qT = [res.tile([128, T], F32R, tag=f"qT{h}", name=f"qT{h}")
                  for h in range(HL)]
            kT = [res.tile([128, T], F32R, tag=f"kT{h}", name=f"kT{h}")
                  for h in range(HL)]
            V = [res.tile([128, CL], BF16, tag=f"V{i}", name=f"V{i}")
                 for i in range(T // 128)]

            ones32 = res.tile([128, 1], F32, tag="ones32")
            nc.gpsimd.memset(ones32[:], 1.0)
            ones = res.tile([128, 1], BF16, tag="ones")
            nc.vector.tensor_copy(ones[:], ones32[:])

            # ---------------- phase 1: QKV projection ----------------
            with tc.tile_pool(name="wpool", bufs=1) as wpool, \
                 tc.tile_pool(name="xpool", bufs=1) as xpool, \
                 tc.tile_pool(name="ps1", bufs=3, space="PSUM") as ps1:
                # first k-tile of qk weights first, then x chunk 0, then rest
                wqk = [[None] * 4 for _ in range(KT)]

                def load_wqk(k):
                    for m in range(4):
                        t_ = wpool.tile([128, 128], F32R,
                                        tag=f"wqk{k}_{m}", name=f"wqk{k}_{m}")
                        nc.sync.dma_start(
                            t_[:],
                            wqkT_d.ap()[k * 128:(k + 1) * 128,
                                        m * 128:(m + 1) * 128].bitcast(F32R),
                        )
                        wqk[k][m] = t_

                load_wqk(0)

                def load_x_chunk(j):
                    xt = []
                    for k in range(KT):
                        t_ = xpool.tile([128, TC1], F32R, tag=f"x{k}",
                                        name=f"x{j}_{k}",
                                        bufs=2 if k < 8 else 1)
                        nc.sync.dma_start(
                            t_[:],
                            xT_d.ap()[k * 128:(k + 1) * 128,
                                      j * TC1:(j + 1) * TC1].bitcast(F32R),
                        )
                        xt.append(t_)
                    return xt

                xt0 = load_x_chunk(0)
                for k in range(1, KT):
                    load_wqk(k)
                wv = []
                for k in range(KT):
                    t_ = wpool.tile([128, CL], F32R, tag=f"wv{k}", name=f"wv{k}")
                    nc.sync.dma_start(
                        t_[:],
                        wvT_d.ap()[k * 128:(k + 1) * 128, :].bitcast(F32R),
                    )
                    wv.append(t_)

                for j in range(NC1):
                    xt = xt0 if j == 0 else load_x_chunk(j)
                    # qT/kT for both heads: out[d, t] accumulated over c
                    for m in range(4):
                        pq = ps1.tile([128, TC1], F32, tag="pqk")
                        for k in range(KT):
                            nc.tensor.matmul(pq[:], wqk[k][m][:], xt[k][:],
                                             start=(k == 0), stop=(k == KT - 1))
                        dest = qT[m] if m < HL else kT[m - HL]
                        nc.scalar.copy(dest[:, j * TC1:(j + 1) * TC1], pq[:])
                    # V: out[t, d] accumulated over c (bf16 storage)
                    for tt in range(TC1 // 128):
                        pv = ps1.tile([128, CL], F32, tag="pv")
                        for k in range(KT):
                            nc.tensor.matmul(
                                pv[:],
                                xt[k][:, tt * 128:(tt + 1) * 128],
                                wv[k][:],
                                start=(k == 0), stop=(k == KT - 1))
                        nc.vector.tensor_copy(
                            V[j * (TC1 // 128) + tt][:], pv[:])

            # ---------------- phase 2: attention ----------------
            with tc.tile_pool(name="ph2", bufs=4) as p2, \
                 tc.tile_pool(name="ph2s", bufs=2) as p2s, \
                 tc.tile_pool(name="a2s", bufs=2) as a2s, \
                 tc.tile_pool(name="ps2s", bufs=3, space="PSUM") as ps2s, \
                 tc.tile_pool(name="ps2o", bufs=2, space="PSUM") as ps2o, \
                 tc.tile_pool(name="ps2d", bufs=2, space="PSUM") as ps2d:
                for h in range(HL):
                    for j in range(NC2):
                        nk = (j + 1) * (TC2 // 128)  # causal: s tiles 0..nk-1
                        po = ps2o.tile([128, TC2], F32, tag="po")
                        pd = ps2d.tile([1, TC2], F32, tag="pd")
                        for k in range(nk):
                            ps = ps2s.tile([128, TC2], F32, tag="ps")
                            nc.tensor.matmul(ps[:],
                                             kT[h][:, k * 128:(k + 1) * 128],
                                             qT[h][:, j * TC2:(j + 1) * TC2],
                                             start=True, stop=True)
                            e = p2.tile([128, TC2], BF16, tag="e")
                            nc.scalar.activation(e[:], ps[:],
                                                 mybir.ActivationFunctionType.Exp,
                                                 scale=SCALE)
                            if k * 128 + 127 > j * TC2:
                                # diagonal tile: keep where t >= s,
                                # s = 128k + p, t = TC2*j + f
                                nc.gpsimd.affine_select(
                                    out=e[:], in_=e[:],
                                    compare_op=mybir.AluOpType.is_ge,
                                    fill=0.0,
                                    base=TC2 * j - 128 * k,
                                    channel_multiplier=-1,
                                    pattern=[[1, TC2]],
                                )
                            nc.tensor.matmul(pd[:], ones[:], e[:],
                                             start=(k == 0), stop=(k == nk - 1))
                            nc.tensor.matmul(po[:],
                                             V[k][:, h * 128:(h + 1) * 128],
                                             e[:],
                                             start=(k == 0), stop=(k == nk - 1))
                        recip = p2s.tile([1, TC2], F32, tag="recip")
                        nc.vector.reciprocal(recip[:], pd[:])
                        r128 = p2s.tile([128, TC2], F32, tag="r128")
                        nc.gpsimd.partition_broadcast(r128[:], recip[:])
                        att = a2s.tile([128, TC2], BF16, tag="att")
                        nc.vector.tensor_mul(att[:], po[:], r128[:])
                        nc.sync.dma_start(a2a_in[h][j, :, :], att[:])
                    # fire this head's A2A as soon as its chunks are written;
                    # head 0's collective overlaps head 1's compute
                    nc.gpsimd.collective_compute(
                        "AllToAll",
                        mybir.AluOpType.bypass,
                        ins=[a2a_in[h].opt()],
                        outs=[a2a_out[h].opt()],
                        replica_groups=[list(range(W))],
                    )

            # ---------------- phase 3: output projection (bf16) ----------------
            with tc.tile_pool(name="p3a", bufs=1) as p3a, \
                 tc.tile_pool(name="p3w", bufs=3) as p3w, \
                 tc.tile_pool(name="p3wb", bufs=20) as p3wb, \
                 tc.tile_pool(name="p3o", bufs=2) as p3o, \
                 tc.tile_pool(name="ps3", bufs=2, space="PSUM") as ps3:
                attn = []
                for kc in range(KT):
                    t_ = p3a.tile([128, TL], BF16, tag=f"at{kc}", name=f"at{kc}")
                    nc.sync.dma_start(t_[:], a2a_out[kc % HL][kc // HL, :, :])
                    attn.append(t_)
                for oc in range(C // 512):
                    wp_oc = []
                    for kc in range(KT):
                        w32 = p3w.tile([128, 512], F32, tag="wp32",
                                       name=f"wp32_{oc}_{kc}")
                        nc.sync.dma_start(
                            w32[:],
                            wpT_d.ap()[kc * 128:(kc + 1) * 128,
                                       oc * 512:(oc + 1) * 512],
                        )
                        wb = p3wb.tile([128, 512], BF16, tag="wp",
                                       name=f"wp{oc}_{kc}")
                        nc.vector.tensor_copy(wb[:], w32[:])
                        wp_oc.append(wb)
                    for tt in range(TL // 128):
                        po3 = ps3.tile([128, 512], F32, tag="po3")
                        for kc in range(KT):
                            nc.tensor.matmul(po3[:],
                                             attn[kc][:, tt * 128:(tt + 1) * 128],
                                             wp_oc[kc][:],
                                             start=(kc == 0), stop=(kc == KT - 1))
                        ob = p3o.tile([128, 512], F32, tag="ob")
                        nc.scalar.copy(ob[:], po3[:])
                        nc.sync.dma_start(
                            out_d.ap()[tt * 128:(tt + 1) * 128,
                                       oc * 512:(oc + 1) * 512], ob[:])

    nc.compile()
    return nc


def _maybe_install_trace_hook():
    try:
        import antenv
        from trn_agent_boot.trn_boot import _ntff_profile_via_ctypes
        hook = _ntff_profile_via_ctypes("/opt/axon/libaxon_pjrt.so")
        mod = types.ModuleType("antenv.axon_hooks")
        mod.get_axon_ntff_profile_hook = lambda: hook
        mod.set_axon_ntff_profile_hook = lambda h: None
        sys.modules["antenv.axon_hooks"] = mod
        antenv.axon_hooks = mod
        return True
    except Exception:
        return False


def kernel(x, w_attn, w_proj):
    x = np.ascontiguousarray(x, dtype=np.float32)
    w_attn = np.ascontiguousarray(w_attn, dtype=np.float32)
    w_proj = np.ascontiguousarray(w_proj, dtype=np.float32)

    if "nc" not in _cache:
        _cache["nc"] = _build()
    nc = _cache["nc"]

    xT = np.ascontiguousarray(x.T)
    wpT = np.ascontiguousarray(w_proj.T)
    in_maps = []
    for c in range(W):
        r0 = CL * c
        wqk = np.concatenate(
            [w_attn[r0:r0 + CL], w_attn[C + r0:C + r0 + CL]], axis=0)
        wqkT = np.ascontiguousarray(wqk.T)
        wvT = np.ascontiguousarray(w_attn[2 * C + r0:2 * C + r0 + CL].T)
        in_maps.append({"xT": xT, "wqkT": wqkT, "wvT": wvT, "wpT": wpT})

    trace = TRACE and _maybe_install_trace_hook()
    res = run_bass_kernel_spmd(nc, in_maps, list(range(W)), trace=trace)
    LAST_RESULT["exec_time_ns"] = res.exec_time_ns

    return np.concatenate([res.results[c]["out"] for c in range(W)], axis=0)
